# revision 19
# baseline (speedup 1.0000x reference)
"""Trainium2 Bass kernel for CustomBSplineLayer.

Computes out[b,o] = sum_{i,g} spline(x)[b,i,g] * coef[o,i,g] where
spline is an order-3 (cubic) B-spline basis on uniform knots applied to
tanh(x).

Math (validated against the reference recursion):
  u = 3.5*tanh(x) + 3.5              in (0, 7)
  basis_g(u) = M4(u - g)             cardinal cubic B-spline, g = 0..7
  M4(s) = (relu(2-|s-2|)^3 - 4*relu(1-|s-2|)^3) / 6
Plane g=7 is identically zero on (0,7), so only 7 of 8 planes
contribute (K = 7*1024 = 7168 per batch row).

With p = relu(2*C6 - C6*|s-2|) and q = relu(KQ*p - C46) (C6^3 = 1/6,
C46^3 = 4/6, KQ = C46/C6), M4 = p^3 - q^3 exactly.

Per-core layout (data-parallel over batch, 8 cores x 512 rows):
  - host pre-transposes x so tiles arrive as [i partitions, b cols];
    basis planes in [i, b] layout feed the PE directly as the stationary
    (lhsT) operand; coef (host-rearranged to [g, i, o], bf16) is the
    moving operand; out accumulates in PSUM as [b, o] across 56 k-tiles.
  - both matmul operands are bf16: the PE streams bf16 moving operands
    at ~2 cols/cycle (vs 1 for tf32), and coef DMA traffic halves.
  - basis pipeline per i-tile: ACT does only tanh; each plane is TWO
    fused custom DVE ops (8-slice single-pass ALU programs):
      OPA: p = relu(2*C6 - |t*(3.5*C6) + C6*(1.5-g)|)
      OPB: s = p^3 - relu(p*KQ - C46)^3   (written as bf16)
  - emission is software-pipelined: planes for i-tile N+1 are emitted
    before the matmuls of i-tile N so the PE never waits; matmuls
    visit the 8 PSUM banks round-robin (g-outer) so each bank's
    accumulate retires before its next write; the last i-tile runs
    m-outer so each bank pair drains + DMAs while the rest still stream.
"""

import sys

sys.path.insert(0, "/opt/trn_rl_repo")

import numpy as np
import ml_dtypes
from contextlib import ExitStack

import concourse.bass as bass
import concourse.tile as tile
from concourse import bacc, mybir
from concourse.bass_utils import run_bass_kernel_spmd
import concourse.dve_ops as dve_ops
from concourse.dve_spec import (
    Spec,
    Src0,
    C0,
    C1,
    C2,
    Zero,
    relu,
    maxx,
    sq,
    lower,
    _has_src1,
)
from concourse.dve_uop import DveOpSpec

F32 = mybir.dt.float32
F32R = mybir.dt.float32r
BF16 = mybir.dt.bfloat16
AF = mybir.ActivationFunctionType
OP = mybir.AluOpType

B, I, O = 4096, 1024, 1024
G = 7                    # active basis planes (plane 7 == 0)
NCORES = 8
BC = B // NCORES         # 512 batch rows per core
IT = I // 128            # 8 i-tiles
KT = IT * G              # 56 k-tiles of 128
WIDE = G * BC            # 3584

C6 = float(6.0 ** (-1.0 / 3.0))          # folds the 1/6 into p
C46 = float((4.0 / 6.0) ** (1.0 / 3.0))  # folds the 4/6 into q
KQ = float(C46 / C6)                     # q = relu(KQ*p - C46)

MM_DT = BF16

LAST_RESULT = None  # BassKernelResults of the most recent run (for test.py)

_cache = {}


def _tf32_round(a: np.ndarray) -> np.ndarray:
    """Round fp32 to tf32 (10-bit mantissa), round-to-nearest-even."""
    bits = np.ascontiguousarray(a, dtype=np.float32).view(np.uint32).copy()
    lsb = (bits >> np.uint32(13)) & np.uint32(1)
    bits += np.uint32(0xFFF) + lsb
    bits &= np.uint32(0xFFFFE000)
    return bits.view(np.float32)


def _register_op(name: str, spec: Spec) -> "dve_ops.DveOp":
    """Register a custom DVE op at runtime (concourse keys the per-NEFF
    uop table and CoreSim reference off these module-level registries)."""
    for op in dve_ops.OPS:
        if op.name == name:
            return op
    row = dve_ops._CUSTOM_DVE_ROW_BASE + len(dve_ops.OPS)
    assert row < 0x20, "custom-DVE opcode rows exhausted"
    shas = {}
    for ver in ("v3", "v4"):
        try:
            uops = lower(spec, ver=ver)
            shas[ver] = DveOpSpec(
                name=name, opcode=row, uops=uops, rd1_en=_has_src1(spec)
            ).sha(ver)
        except Exception:
            pass
    op = dve_ops.DveOp(name, spec, subdim=False, uops_sha=shas)
    dve_ops.OPS.append(op)
    dve_ops.CUSTOM_DVE_SPECS[name] = spec
    dve_ops._SUB_OPCODE_FOR_NAME[name] = row
    return op


# opA: p = relu(imm2 - |in0*s0 + s1|)
_w = Src0 * C0 + C1
OPA = _register_op(
    "BSPLINE_P_ANT",
    Spec(
        body=relu(C2 - maxx(_w, Zero - _w)),
        reference=lambda in0, in1, s0, s1, imm2: np.maximum(
            imm2 - np.abs(in0.astype(np.float32) * s0 + s1), 0.0
        ).astype(np.float32),
    ),
)

# opB: s = in0^3 - relu(in0*s0 - s1)^3
_q = relu(Src0 * C0 - C1)
OPB = _register_op(
    "BSPLINE_CUBE_ANT",
    Spec(
        body=sq(Src0) * Src0 - sq(_q) * _q,
        reference=lambda in0, in1, s0, s1, imm2: (
            in0.astype(np.float32) ** 3
            - np.maximum(in0.astype(np.float32) * s0 - s1, 0.0) ** 3
        ).astype(np.float32),
    ),
)


def _build_nc(repeats: int = 1):
    nc = bacc.Bacc("TRN2", target_bir_lowering=False, debug=False)
    xT = nc.dram_tensor("xT", [I, BC], F32, kind="ExternalInput").ap()
    coefT = nc.dram_tensor("coefT", [G, I, O], MM_DT, kind="ExternalInput").ap()
    y = nc.dram_tensor("y", [BC, O], F32, kind="ExternalOutput").ap()

    with tile.TileContext(nc) as tc, ExitStack() as ctx:
        xt_pool = ctx.enter_context(tc.tile_pool(name="xt", bufs=3))
        t_pool = ctx.enter_context(tc.tile_pool(name="t", bufs=4))
        pw_pool = ctx.enter_context(tc.tile_pool(name="pw", bufs=2))
        pc_pool = ctx.enter_context(tc.tile_pool(name="pc", bufs=2))
        spl_pool = ctx.enter_context(tc.tile_pool(name="spl", bufs=4))
        rhs_pool = ctx.enter_context(tc.tile_pool(name="rhs", bufs=16))
        out_pool = ctx.enter_context(tc.tile_pool(name="ot", bufs=4))
        psum_pool = ctx.enter_context(
            tc.tile_pool(name="psum", bufs=1, space=bass.MemorySpace.PSUM)
        )

        # 8 PSUM banks: [m-tile 0..3] x [o-half 0..1], each [128, 512] f32
        psum = [
            [
                psum_pool.tile([128, 512], F32, tag=f"ps{m}_{h}", name=f"ps{m}_{h}")
                for h in range(2)
            ]
            for m in range(4)
        ]

        def emit_front(rep, it):
            """DMA + tanh for one i-tile; returns the t tile."""
            xt = xt_pool.tile([128, BC], F32, tag="xt", name=f"xt{rep}_{it}")
            nc.sync.dma_start(xt[:], xT[it * 128 : (it + 1) * 128, :])
            t = t_pool.tile([128, BC], F32, tag="t", name=f"t{rep}_{it}")
            nc.scalar.activation(t[:], xt[:], AF.Tanh)
            return t

        def emit_planes(rep, it, t):
            """One i-tile's basis planes + rhs DMAs, interleaved per plane
            ([dma, opA, opB] per g) so each plane's spl slice completes as
            early as possible."""
            pw = pw_pool.tile([128, WIDE], F32, tag="pw", name=f"pw{rep}_{it}")
            spl = spl_pool.tile([128, WIDE], MM_DT, tag="spl", name=f"spl{rep}_{it}")
            rhs_g = {}
            for g in range(G):
                rhs = rhs_pool.tile(
                    [128, O], MM_DT, tag="rhs", name=f"rhs{rep}_{it}_{g}"
                )
                nc.sync.dma_start(rhs[:], coefT[g, it * 128 : (it + 1) * 128, :])
                rhs_g[g] = rhs
                sl = slice(g * BC, (g + 1) * BC)
                nc.vector._custom_dve(
                    OPA, out=pw[:, sl], in0=t[:],
                    s0=3.5 * C6, s1=C6 * (1.5 - g), imm2=2.0 * C6,
                )
                nc.vector._custom_dve(OPB, out=spl[:, sl], in0=pw[:, sl], s0=KQ, s1=C46)
            return spl, rhs_g

        def emit_mms(splrhs, kt):
            """Stage M: the 56 matmuls of one i-tile, k-major (g-outer):
            the 8 PSUM banks are visited round-robin so each bank has 7
            matmuls of spacing to retire its accumulate before the next
            write (back-to-back same-bank accumulates stall on writeback)."""
            spl, rhs_g = splrhs
            for g in range(G):
                first = kt == 0
                last = kt == KT - 1
                for m in range(4):
                    lhsT = spl[:, g * BC + m * 128 : g * BC + (m + 1) * 128]
                    for h in range(2):
                        nc.tensor.matmul(
                            psum[m][h][:],
                            lhsT,
                            rhs_g[g][:, h * 512 : (h + 1) * 512],
                            start=first,
                            stop=last,
                        )
                kt += 1
            return kt

        def emit_first_tile_chunked(rep):
            """i-tile 0 of rep 0 in 128-col chunks, plane-major: the first
            matmul can issue ~1.5us into the kernel and the PE ramps while
            coef DMA streams.  All planes use the DVE path here."""
            # HAM warm-up: the PE clock-gate sits at 1.2 GHz until ~3.4us of
            # sustained activity.  Fill the otherwise-idle window before the
            # first real matmul with dependency-free dummy matmuls so the
            # real stream starts (mostly) warm.
            warm = pc_pool.tile([128, 64], MM_DT, tag="warm", name="warm0")
            nc.gpsimd.memset(warm[:], 0.0)
            for i in range(18):
                nc.tensor.matmul(
                    psum[0][0][:64, :64], warm[:], warm[:], start=True, stop=True
                )
            xt = xt_pool.tile([128, BC], F32, tag="xt", name=f"xt{rep}_0")
            t = t_pool.tile([128, BC], F32, tag="t", name=f"t{rep}_0")
            for c in range(4):
                sl = slice(c * 128, (c + 1) * 128)
                nc.sync.dma_start(xt[:, sl], xT[0:128, c * 128 : (c + 1) * 128])
                nc.scalar.activation(t[:, sl], xt[:, sl], AF.Tanh)
            spl = spl_pool.tile([128, WIDE], MM_DT, tag="spl", name=f"spl{rep}_0")
            rhs_g = {}
            for g in range(G):
                r = rhs_pool.tile([128, O], MM_DT, tag="rhs", name=f"rhs{rep}_0_{g}")
                nc.sync.dma_start(r[:], coefT[g, 0:128, :])
                rhs_g[g] = r
                for c in range(4):
                    sl = slice(c * 128, (c + 1) * 128)
                    p = pc_pool.tile([128, 128], F32, tag="pc", name=f"pc{rep}_{g}_{c}")
                    nc.vector._custom_dve(
                        OPA, out=p[:], in0=t[:, sl],
                        s0=3.5 * C6, s1=C6 * (1.5 - g), imm2=2.0 * C6,
                    )
                    osl = slice(g * BC + c * 128, g * BC + (c + 1) * 128)
                    nc.vector._custom_dve(OPB, out=spl[:, osl], in0=p[:], s0=KQ, s1=C46)
                    for h in range(2):
                        nc.tensor.matmul(
                            psum[c][h][:],
                            spl[:, osl],
                            rhs_g[g][:, h * 512 : (h + 1) * 512],
                            start=(g == 0),
                            stop=False,
                        )

        next_front = None  # pre-emitted tanh tile for the next rep's i-tile 0
        for _rep in range(repeats):
            if _rep == 0:
                emit_first_tile_chunked(0)
                kt = G
                pend = emit_planes(0, 1, emit_front(0, 1))
                start_it = 2
            else:
                kt = 0
                pend = emit_planes(_rep, 0, next_front)
                next_front = None
                start_it = 1
            for it in range(start_it, IT):
                t = emit_front(_rep, it)
                nxt = emit_planes(_rep, it, t)
                kt = emit_mms(pend, kt)
                pend = nxt
            if _rep + 1 < repeats:
                next_front = emit_front(_rep + 1, 0)
            # Last i-tile: matmuls m-outer so bank m finishes 14*(3-m)
            # matmuls early and its PSUM drain + y DMA overlap the remaining
            # stream (within one bank the 7 accumulates stay g-spaced via
            # the h-interleave).  The next rep's tanh was emitted before the
            # drain copies so the ACT FIFO doesn't head-of-line-block it.
            spl, rhs_g = pend
            for m in range(4):
                for g in range(G):
                    lhsT = spl[:, g * BC + m * 128 : g * BC + (m + 1) * 128]
                    for h in range(2):
                        nc.tensor.matmul(
                            psum[m][h][:],
                            lhsT,
                            rhs_g[g][:, h * 512 : (h + 1) * 512],
                            start=False,
                            stop=(g == G - 1),
                        )
                ot = out_pool.tile([128, O], F32, tag="ot", name=f"ot{_rep}_{m}")
                for h in range(2):
                    nc.scalar.copy(ot[:, h * 512 : (h + 1) * 512], psum[m][h][:])
                nc.sync.dma_start(y[m * 128 : (m + 1) * 128, :], ot[:])

    nc.compile()
    return nc


def kernel(x: np.ndarray, coef: np.ndarray) -> np.ndarray:
    global LAST_RESULT
    x = np.asarray(x, dtype=np.float32)
    coef = np.asarray(coef, dtype=np.float32)
    assert x.shape == (B, I) and coef.shape == (O, I, 8)

    if "nc" not in _cache:
        _cache["nc"] = _build_nc()
    nc = _cache["nc"]

    xT = np.ascontiguousarray(x.T)  # [I, B]
    coefT = np.ascontiguousarray(
        coef.transpose(2, 1, 0)[:G].astype(ml_dtypes.bfloat16)
    )  # [7, I, O] bf16
    in_maps = [
        {
            "xT": np.ascontiguousarray(xT[:, c * BC : (c + 1) * BC]),
            "coefT": coefT,
        }
        for c in range(NCORES)
    ]
    res = run_bass_kernel_spmd(nc, in_maps, list(range(NCORES)))
    LAST_RESULT = res
    out = np.concatenate([res.results[c]["y"] for c in range(NCORES)], axis=0)
    return np.ascontiguousarray(out.astype(np.float32))


if __name__ == "__main__":
    rng = np.random.default_rng(0)
    x = rng.standard_normal((B, I), dtype=np.float32)
    coef = rng.standard_normal((O, I, 8), dtype=np.float32) * 0.1
    out = kernel(x, coef)
    print("out", out.shape, out.dtype, float(np.abs(out).max()))


# revision 21
# speedup vs baseline: 1.1107x; 1.1107x over previous
"""Trainium2 Bass kernel for CustomBSplineLayer.

Computes out[b,o] = sum_{i,g} spline(x)[b,i,g] * coef[o,i,g] where
spline is an order-3 (cubic) B-spline basis on uniform knots applied to
tanh(x).

Math (validated against the reference recursion):
  u = 3.5*tanh(x) + 3.5              in (0, 7)
  basis_g(u) = M4(u - g)             cardinal cubic B-spline, g = 0..7
  M4(s) = (relu(2-|s-2|)^3 - 4*relu(1-|s-2|)^3) / 6
Plane g=7 is identically zero on (0,7), so only 7 of 8 planes
contribute (K = 7*1024 = 7168 per batch row).

With p = relu(2*C6 - C6*|s-2|) and q = relu(KQ*p - C46) (C6^3 = 1/6,
C46^3 = 4/6, KQ = C46/C6), M4 = p^3 - q^3 exactly.

Per-core layout (data-parallel over batch, 8 cores x 512 rows):
  - host pre-transposes x so tiles arrive as [i partitions, b cols];
    basis planes in [i, b] layout feed the PE directly as the stationary
    (lhsT) operand; coef (host-rearranged to [g, i, o], bf16) is the
    moving operand; out accumulates in PSUM as [b, o] across 56 k-tiles.
  - both matmul operands are bf16: the PE streams bf16 moving operands
    at ~2 cols/cycle (vs 1 for tf32), and coef DMA traffic halves.
  - basis pipeline per i-tile: ACT does only tanh; each plane is TWO
    fused custom DVE ops (8-slice single-pass ALU programs):
      OPA: p = relu(2*C6 - |t*(3.5*C6) + C6*(1.5-g)|)
      OPB: s = p^3 - relu(p*KQ - C46)^3   (written as bf16)
  - emission is software-pipelined: planes for i-tile N+1 are emitted
    before the matmuls of i-tile N so the PE never waits; matmuls
    visit the 8 PSUM banks round-robin (g-outer) so each bank's
    accumulate retires before its next write; the last i-tile runs
    m-outer so each bank pair drains + DMAs while the rest still stream.
"""

import sys

sys.path.insert(0, "/opt/trn_rl_repo")

import numpy as np
import ml_dtypes
from contextlib import ExitStack

import concourse.bass as bass
import concourse.tile as tile
from concourse import bacc, mybir
from concourse.bass_utils import run_bass_kernel_spmd
import concourse.dve_ops as dve_ops
from concourse.dve_spec import (
    Spec,
    Src0,
    C0,
    C1,
    C2,
    Zero,
    relu,
    maxx,
    sq,
    lower,
    _has_src1,
)
from concourse.dve_uop import DveOpSpec

F32 = mybir.dt.float32
F32R = mybir.dt.float32r
BF16 = mybir.dt.bfloat16
AF = mybir.ActivationFunctionType
OP = mybir.AluOpType

B, I, O = 4096, 1024, 1024
G = 7                    # active basis planes (plane 7 == 0)
NCORES = 8
BC = B // NCORES         # 512 batch rows per core
IT = I // 128            # 8 i-tiles
KT = IT * G              # 56 k-tiles of 128
WIDE = G * BC            # 3584

C6 = float(6.0 ** (-1.0 / 3.0))          # folds the 1/6 into p
C46 = float((4.0 / 6.0) ** (1.0 / 3.0))  # folds the 4/6 into q
KQ = float(C46 / C6)                     # q = relu(KQ*p - C46)

MM_DT = BF16

LAST_RESULT = None  # BassKernelResults of the most recent run (for test.py)

_cache = {}


def _tf32_round(a: np.ndarray) -> np.ndarray:
    """Round fp32 to tf32 (10-bit mantissa), round-to-nearest-even."""
    bits = np.ascontiguousarray(a, dtype=np.float32).view(np.uint32).copy()
    lsb = (bits >> np.uint32(13)) & np.uint32(1)
    bits += np.uint32(0xFFF) + lsb
    bits &= np.uint32(0xFFFFE000)
    return bits.view(np.float32)


def _register_op(name: str, spec: Spec) -> "dve_ops.DveOp":
    """Register a custom DVE op at runtime (concourse keys the per-NEFF
    uop table and CoreSim reference off these module-level registries)."""
    for op in dve_ops.OPS:
        if op.name == name:
            return op
    row = dve_ops._CUSTOM_DVE_ROW_BASE + len(dve_ops.OPS)
    assert row < 0x20, "custom-DVE opcode rows exhausted"
    shas = {}
    for ver in ("v3", "v4"):
        try:
            uops = lower(spec, ver=ver)
            shas[ver] = DveOpSpec(
                name=name, opcode=row, uops=uops, rd1_en=_has_src1(spec)
            ).sha(ver)
        except Exception:
            pass
    op = dve_ops.DveOp(name, spec, subdim=False, uops_sha=shas)
    dve_ops.OPS.append(op)
    dve_ops.CUSTOM_DVE_SPECS[name] = spec
    dve_ops._SUB_OPCODE_FOR_NAME[name] = row
    return op


# opA: p = relu(imm2 - |in0*s0 + s1|)
_w = Src0 * C0 + C1
OPA = _register_op(
    "BSPLINE_P_ANT",
    Spec(
        body=relu(C2 - maxx(_w, Zero - _w)),
        reference=lambda in0, in1, s0, s1, imm2: np.maximum(
            imm2 - np.abs(in0.astype(np.float32) * s0 + s1), 0.0
        ).astype(np.float32),
    ),
)

# opB: s = in0^3 - relu(in0*s0 - s1)^3
_q = relu(Src0 * C0 - C1)
OPB = _register_op(
    "BSPLINE_CUBE_ANT",
    Spec(
        body=sq(Src0) * Src0 - sq(_q) * _q,
        reference=lambda in0, in1, s0, s1, imm2: (
            in0.astype(np.float32) ** 3
            - np.maximum(in0.astype(np.float32) * s0 - s1, 0.0) ** 3
        ).astype(np.float32),
    ),
)


def _build_nc(repeats: int = 1):
    nc = bacc.Bacc("TRN2", target_bir_lowering=False, debug=False)
    xT = nc.dram_tensor("xT", [I, BC], F32, kind="ExternalInput").ap()
    coefT = nc.dram_tensor("coefT", [I, G * O], MM_DT, kind="ExternalInput").ap()
    y = nc.dram_tensor("y", [BC, O], F32, kind="ExternalOutput").ap()

    with tile.TileContext(nc) as tc, ExitStack() as ctx:
        xt_pool = ctx.enter_context(tc.tile_pool(name="xt", bufs=3))
        t_pool = ctx.enter_context(tc.tile_pool(name="t", bufs=4))
        pw_pool = ctx.enter_context(tc.tile_pool(name="pw", bufs=2))
        pc_pool = ctx.enter_context(tc.tile_pool(name="pc", bufs=2))
        spl_pool = ctx.enter_context(tc.tile_pool(name="spl", bufs=4))
        rhs_pool = ctx.enter_context(tc.tile_pool(name="rhs", bufs=3))
        out_pool = ctx.enter_context(tc.tile_pool(name="ot", bufs=4))
        psum_pool = ctx.enter_context(
            tc.tile_pool(name="psum", bufs=1, space=bass.MemorySpace.PSUM)
        )

        # 8 PSUM banks: [m-tile 0..3] x [o-half 0..1], each [128, 512] f32
        psum = [
            [
                psum_pool.tile([128, 512], F32, tag=f"ps{m}_{h}", name=f"ps{m}_{h}")
                for h in range(2)
            ]
            for m in range(4)
        ]

        def emit_front(rep, it):
            """DMA + tanh for one i-tile; returns the t tile."""
            xt = xt_pool.tile([128, BC], F32, tag="xt", name=f"xt{rep}_{it}")
            nc.sync.dma_start(xt[:], xT[it * 128 : (it + 1) * 128, :])
            t = t_pool.tile([128, BC], F32, tag="t", name=f"t{rep}_{it}")
            nc.scalar.activation(t[:], xt[:], AF.Tanh)
            return t

        def emit_planes(rep, it, t):
            """One i-tile's basis planes + rhs DMAs, interleaved per plane
            ([dma, opA, opB] per g) so each plane's spl slice completes as
            early as possible."""
            pw = pw_pool.tile([128, WIDE], F32, tag="pw", name=f"pw{rep}_{it}")
            spl = spl_pool.tile([128, WIDE], MM_DT, tag="spl", name=f"spl{rep}_{it}")
            # all 7 planes' coef blocks in ONE contiguous 2D DMA (host
            # stores coef as [i, g, o] so an i-tile's planes are one block)
            rhs = rhs_pool.tile([128, G * O], MM_DT, tag="rhs", name=f"rhs{rep}_{it}")
            nc.sync.dma_start(rhs[:], coefT[it * 128 : (it + 1) * 128, :])
            for g in range(G):
                sl = slice(g * BC, (g + 1) * BC)
                nc.vector._custom_dve(
                    OPA, out=pw[:, sl], in0=t[:],
                    s0=3.5 * C6, s1=C6 * (1.5 - g), imm2=2.0 * C6,
                )
                nc.vector._custom_dve(OPB, out=spl[:, sl], in0=pw[:, sl], s0=KQ, s1=C46)
            return spl, rhs

        def emit_mms(splrhs, kt):
            """Stage M: the 56 matmuls of one i-tile, k-major (g-outer):
            the 8 PSUM banks are visited round-robin so each bank has 7
            matmuls of spacing to retire its accumulate before the next
            write (back-to-back same-bank accumulates stall on writeback)."""
            spl, rhs = splrhs
            for g in range(G):
                first = kt == 0
                last = kt == KT - 1
                for m in range(4):
                    lhsT = spl[:, g * BC + m * 128 : g * BC + (m + 1) * 128]
                    for h in range(2):
                        nc.tensor.matmul(
                            psum[m][h][:],
                            lhsT,
                            rhs[:, g * O + h * 512 : g * O + (h + 1) * 512],
                            start=first,
                            stop=last,
                        )
                kt += 1
            return kt

        def emit_first_tile_chunked(rep):
            """i-tile 0 of rep 0 in 128-col chunks, plane-major: the first
            matmul can issue ~1.5us into the kernel and the PE ramps while
            coef DMA streams.  All planes use the DVE path here."""
            # HAM warm-up: the PE clock-gate sits at 1.2 GHz until ~3.4us of
            # sustained activity.  Fill the otherwise-idle window before the
            # first real matmul with dependency-free dummy matmuls so the
            # real stream starts (mostly) warm.
            warm = pc_pool.tile([128, 64], MM_DT, tag="warm", name="warm0")
            nc.gpsimd.memset(warm[:], 0.0)
            for i in range(18):
                nc.tensor.matmul(
                    psum[0][0][:64, :64], warm[:], warm[:], start=True, stop=True
                )
            xt = xt_pool.tile([128, BC], F32, tag="xt", name=f"xt{rep}_0")
            t = t_pool.tile([128, BC], F32, tag="t", name=f"t{rep}_0")
            for c in range(4):
                sl = slice(c * 128, (c + 1) * 128)
                nc.sync.dma_start(xt[:, sl], xT[0:128, c * 128 : (c + 1) * 128])
                nc.scalar.activation(t[:, sl], xt[:, sl], AF.Tanh)
            spl = spl_pool.tile([128, WIDE], MM_DT, tag="spl", name=f"spl{rep}_0")
            rhs = rhs_pool.tile([128, G * O], MM_DT, tag="rhs", name=f"rhs{rep}_0")
            nc.sync.dma_start(rhs[:], coefT[0:128, :])
            for g in range(G):
                for c in range(4):
                    sl = slice(c * 128, (c + 1) * 128)
                    p = pc_pool.tile([128, 128], F32, tag="pc", name=f"pc{rep}_{g}_{c}")
                    nc.vector._custom_dve(
                        OPA, out=p[:], in0=t[:, sl],
                        s0=3.5 * C6, s1=C6 * (1.5 - g), imm2=2.0 * C6,
                    )
                    osl = slice(g * BC + c * 128, g * BC + (c + 1) * 128)
                    nc.vector._custom_dve(OPB, out=spl[:, osl], in0=p[:], s0=KQ, s1=C46)
                    for h in range(2):
                        nc.tensor.matmul(
                            psum[c][h][:],
                            spl[:, osl],
                            rhs[:, g * O + h * 512 : g * O + (h + 1) * 512],
                            start=(g == 0),
                            stop=False,
                        )

        next_front = None  # pre-emitted tanh tile for the next rep's i-tile 0
        for _rep in range(repeats):
            if _rep == 0:
                emit_first_tile_chunked(0)
                kt = G
                pend = emit_planes(0, 1, emit_front(0, 1))
                start_it = 2
            else:
                kt = 0
                pend = emit_planes(_rep, 0, next_front)
                next_front = None
                start_it = 1
            for it in range(start_it, IT):
                t = emit_front(_rep, it)
                nxt = emit_planes(_rep, it, t)
                kt = emit_mms(pend, kt)
                pend = nxt
            if _rep + 1 < repeats:
                next_front = emit_front(_rep + 1, 0)
            # Last i-tile: matmuls m-outer so bank m finishes 14*(3-m)
            # matmuls early and its PSUM drain + y DMA overlap the remaining
            # stream (within one bank the 7 accumulates stay g-spaced via
            # the h-interleave).  The next rep's tanh was emitted before the
            # drain copies so the ACT FIFO doesn't head-of-line-block it.
            spl, rhs = pend
            for m in range(4):
                for g in range(G):
                    lhsT = spl[:, g * BC + m * 128 : g * BC + (m + 1) * 128]
                    for h in range(2):
                        nc.tensor.matmul(
                            psum[m][h][:],
                            lhsT,
                            rhs[:, g * O + h * 512 : g * O + (h + 1) * 512],
                            start=False,
                            stop=(g == G - 1),
                        )
                ot = out_pool.tile([128, O], F32, tag="ot", name=f"ot{_rep}_{m}")
                for h in range(2):
                    nc.scalar.copy(ot[:, h * 512 : (h + 1) * 512], psum[m][h][:])
                nc.sync.dma_start(y[m * 128 : (m + 1) * 128, :], ot[:])

    nc.compile()
    return nc


def kernel(x: np.ndarray, coef: np.ndarray) -> np.ndarray:
    global LAST_RESULT
    x = np.asarray(x, dtype=np.float32)
    coef = np.asarray(coef, dtype=np.float32)
    assert x.shape == (B, I) and coef.shape == (O, I, 8)

    if "nc" not in _cache:
        _cache["nc"] = _build_nc()
    nc = _cache["nc"]

    xT = np.ascontiguousarray(x.T)  # [I, B]
    coefT = np.ascontiguousarray(
        coef.transpose(1, 2, 0)[:, :G, :].astype(ml_dtypes.bfloat16)
    ).reshape(I, G * O)  # [i, g*o] bf16 — i-tile planes contiguous
    in_maps = [
        {
            "xT": np.ascontiguousarray(xT[:, c * BC : (c + 1) * BC]),
            "coefT": coefT,
        }
        for c in range(NCORES)
    ]
    res = run_bass_kernel_spmd(nc, in_maps, list(range(NCORES)))
    LAST_RESULT = res
    out = np.concatenate([res.results[c]["y"] for c in range(NCORES)], axis=0)
    return np.ascontiguousarray(out.astype(np.float32))


if __name__ == "__main__":
    rng = np.random.default_rng(0)
    x = rng.standard_normal((B, I), dtype=np.float32)
    coef = rng.standard_normal((O, I, 8), dtype=np.float32) * 0.1
    out = kernel(x, coef)
    print("out", out.shape, out.dtype, float(np.abs(out).max()))
